# revision 26
# baseline (speedup 1.0000x reference)
"""NCN link predictor (nn_NCNPredictor_77292231459355) on 8 Trainium2 cores.

Strategy (B-sharded per the sharding hint): the 1024 target pairs are split
128 per core. The host symmetrizes edge_index and ships each core the padded
adjacency rows (a CSR slice, ids compacted to per-core order-preserving
int16 codes, sorted with pads evenly interleaved) of ITS 128 (i, j) target
pairs. On device, each core:
  1. computes c[b,q] = multiplicity of j-neighbor q in i's row (the
     A_i*A_j intersection) with a banded equality pass: sorted placement
     confines matches to a host-verified diagonal band of width D, so the
     compare runs over D overlapping shifts of the i-row (all DVE operands
     2-byte and packed innermost) followed by a packed tree of halving
     adds,
  2. selects the nonzero-weight slots with a top-8 pass on the packed key
     c*512 + gid (gid = compact id of the candidate common neighbor),
  3. builds the weighted selector Wsel[b,s] = sum_k w_k*onehot(gid_k) and
     gathers/combines the needed x rows from an SBUF-resident compact table
     with one transpose + one matmul per 128-row chunk (no indirect DMA /
     GPSIMD anywhere),
  4. computes xcn = A_i.A_j-weighted x sum, xij = x[i]*x[j] (shipped as
     transposed feature-major bf16 blocks), and the MLP head, which runs
     fully transposed (hidden on partitions) so the final W2 contraction is
     a K=128 matmul and the output DMA is one [1, 128] descriptor.
Host concatenates the 8 per-core [128] score slices into the final [1024].

All activation tensors that feed the PE are built directly in lhsT
(feature-major) layout, so the kernel contains no activation transposes.
Per-core NEFF inputs are ~0.5MB total (vs 52.4MB when shipping full x),
loaded in four batched DMAs split across the SP and ACT queue sets.
"""

import numpy as np

N_NODES = 100000
B = 1024
D = 128
DH = 512
N_CORES = 8
BL = B // N_CORES  # 128 pairs per core = SBUF partition dim
TOPK = 8
GID_BASE = 512  # key = c * GID_BASE + gid; gid < n_chunks*128 <= GID_BASE

_compiled_cache: dict = {}


def _padded_rows(src, dst, targets, sentinel):
    """Padded adjacency rows (with multiplicity as repeated entries) of the
    symmetric edge list at `targets` -> float32 [B, S] (S = max degree,
    padded to a multiple of 8, >= 8). Pad slots hold `sentinel`."""
    b = targets.shape[0]
    pos = np.full(N_NODES, -1, np.int32)
    pos[targets] = np.arange(b, dtype=np.int32)
    r = pos[src]
    m = r >= 0
    rows = r[m].astype(np.int64)
    cols = dst[m].astype(np.int64)
    order = np.argsort(rows, kind="stable")
    rows = rows[order]
    cols = cols[order]
    cnt = np.bincount(rows, minlength=b)
    s = max(8, (int(cnt.max()) + 7) // 8 * 8)
    starts = np.zeros(b + 1, np.int64)
    np.cumsum(cnt, out=starts[1:])
    within = np.arange(rows.size, dtype=np.int64) - starts[rows]
    out = np.full((b, s), sentinel, np.float32)
    out[rows, within] = cols.astype(np.float32)
    return out


def _layout(dband, sj, n_chunks):
    """Column offsets for the merged per-core input blocks:
    `hoti` int16 [BL, *] (compact adjacency ids -> equality pass),
    `hotf` f32  [BL, *] (gather ids, per-partition index, b2, iota),
    `cbw`  bf16 [BL, *] (W1),
    `cbx`  bf16 [BL, *] (biases, xiT, xjT, compact x table)."""
    lay = {}
    widths = {}
    for blk, names in [
        ("hoti", [("nis", sj + dband - 1), ("nj", sj)]),
        ("hotf", [("njg", sj), ("b2b", 1), ("pidx", 1),
                  ("iotaf", max(n_chunks * BL, BL))]),
        ("cbw", [
            ("w1a", DH),
            ("w1b", DH),
        ]),
        ("cbx", [
            ("b1c", DH // BL),
            ("w2c", DH // BL),
            ("xiT", D),
            ("xjT", D),
            ("xsm", n_chunks * BL),
        ]),
    ]:
        off = 0
        for name, w in names:
            lay[name] = (blk, off, w)
            off += w
        widths[blk] = off
    return lay, widths


def _build_bass(dband, sj, total_slots, n_chunks, repeat=1):
    """repeat>1 unrolls the whole body N times over the same tiles (serial
    via WAW deps) — used only for amplified wall-clock timing."""
    import concourse.tile as tile
    from concourse import bacc, mybir
    from concourse.ap import AP as _AP

    f32 = mybir.dt.float32
    bf16 = mybir.dt.bfloat16
    i16 = mybir.dt.int16
    i32 = mybir.dt.int32
    eq_dt = bf16  # eq/count values exact in bf16 (0/1 sums <= si < 256)

    lay, widths = _layout(dband, sj, n_chunks)

    nc = bacc.Bacc(
        "TRN2", target_bir_lowering=False, debug=False, num_devices=N_CORES
    )

    hoti_d = nc.dram_tensor(
        "hoti", [BL, widths["hoti"]], i16, kind="ExternalInput"
    ).ap()
    hotf_d = nc.dram_tensor(
        "hotf", [BL, widths["hotf"]], f32, kind="ExternalInput"
    ).ap()
    cbw_d = nc.dram_tensor(
        "cbw", [BL, widths["cbw"]], bf16, kind="ExternalInput"
    ).ap()
    cbx_d = nc.dram_tensor(
        "cbx", [BL, widths["cbx"]], bf16, kind="ExternalInput"
    ).ap()
    out_d = nc.dram_tensor("out", [1, BL], f32, kind="ExternalOutput").ap()

    with tile.TileContext(nc) as tc:
        with (
            tc.tile_pool(name="sb", bufs=2) as sb,
            tc.tile_pool(name="ps", bufs=2, space="PSUM") as ps,
        ):
          for _rep in range(repeat):
            hoti = sb.tile([BL, widths["hoti"]], i16, tag="hoti")
            nc.sync.dma_start(hoti[:], hoti_d[:])
            hotf = sb.tile([BL, widths["hotf"]], f32, tag="hotf")
            nc.sync.dma_start(hotf[:], hotf_d[:])
            cbw = sb.tile([BL, widths["cbw"]], bf16, tag="cbw")
            nc.scalar.dma_start(cbw[:], cbw_d[:])
            cbx = sb.tile([BL, widths["cbx"]], bf16, tag="cbx")
            nc.sync.dma_start(cbx[:], cbx_d[:])

            def bslice(name):
                blk, off, w = lay[name]
                t = {"hoti": hoti, "hotf": hotf, "cbw": cbw,
                     "cbx": cbx}[blk]
                return t[:, off : off + w]

            nis = bslice("nis")
            nji = bslice("nj")
            njg = bslice("njg")
            b2b = bslice("b2b")
            pidx = bslice("pidx")
            iotaf = bslice("iotaf")
            b1c = bslice("b1c")
            w2c = bslice("w2c")
            w1a = bslice("w1a")
            w1b = bslice("w1b")
            xiT = bslice("xiT")
            xjT = bslice("xjT")

            def xsm(m):
                blk, off, w = lay["xsm"]
                return cbx[:, off + m * BL : off + (m + 1) * BL]

            # identity matrix for PE transposes, generated on device
            ident = sb.tile([BL, BL], bf16, tag="ident")
            nc.vector.tensor_scalar(
                out=ident[:], in0=iotaf[:, 0:BL],
                scalar1=pidx, scalar2=None,
                op0=mybir.AluOpType.is_equal,
            )

            # --- intersection counts: c[b,q] = sum_p (NJ[b,q] == NI[b,p]).
            # Both rows are shipped sorted with pads evenly interleaved, so
            # every match lies in a narrow diagonal band of host-verified
            # width D: NIS[t] = NI_slots[t + dmin], and c[b,q] =
            # sum_{d<D} (NJ[b,q] == NIS[b,q+d]). The overlapping strided
            # view keeps every DVE operand 2-byte and packed innermost
            # (fast DVE mode); the d-sum is a packed tree of halving adds
            # (bf16 exact: counts < 256). ---
            eq3 = sb.tile([BL, dband * sj], eq_dt, tag="eq3")
            rot = _AP(
                nis.tensor, nis.offset,
                [[nis.ap[0][0], BL], [1, dband], [1, sj]],
            )
            nc.vector.tensor_tensor(
                out=eq3[:].rearrange("p (d q) -> p d q", q=sj),
                in0=nji[:].unsqueeze(1).broadcast_to([BL, dband, sj]),
                in1=rot,
                op=mybir.AluOpType.is_equal,
            )
            dcur, cur = dband, eq3
            while dcur > 1:
                h = dcur // 2
                nxt = sb.tile([BL, h * sj], eq_dt, tag=f"red_{dcur}")
                cv = cur[:].rearrange("p (d q) -> p d q", q=sj)
                nc.vector.tensor_tensor(
                    out=nxt[:].rearrange("p (d q) -> p d q", q=sj),
                    in0=cv[:, 0:h, :],
                    in1=cv[:, h : 2 * h, :],
                    op=mybir.AluOpType.add,
                )
                if dcur % 2 == 1:
                    nv = nxt[:].rearrange("p (d q) -> p d q", q=sj)
                    nc.vector.tensor_tensor(
                        out=nv[:, 0:1, :],
                        in0=nv[:, 0:1, :],
                        in1=cv[:, 2 * h : 2 * h + 1, :],
                        op=mybir.AluOpType.add,
                    )
                dcur, cur = h, nxt
            cmat = cur  # [BL, sj] counts in eq_dt

            # --- pack keys t = c*512 + gid (gid=0 where c=0 => key=0) ---
            tkey = sb.tile([BL, sj], f32, tag="tkey")
            nc.vector.scalar_tensor_tensor(
                out=tkey[:],
                in0=cmat[:],
                scalar=float(GID_BASE),
                in1=njg[:],
                op0=mybir.AluOpType.mult,
                op1=mybir.AluOpType.add,
            )

            # --- xijT = xiT * xjT  (feature-major: [D, BL]) ---
            xijT = sb.tile([BL, BL], bf16, tag="xijT")
            nc.vector.tensor_mul(out=xijT[:], in0=xiT, in1=xjT)

            # --- MLP first matmuls can start on the xij half immediately.
            # The whole MLP runs transposed (hidden on partitions, pairs on
            # the free axis) so the final W2 dot is a K=128 matmul per chunk
            # and the output DMA is a single [1, BL] descriptor. ---
            nch_h = DH // BL
            psh = ps.tile([BL, DH], f32, tag="psh")
            nc.scalar.copy(
                out=psh[:].rearrange("p (m b) -> p m b", b=BL),
                in_=b1c[:].unsqueeze(2).broadcast_to([BL, nch_h, BL]),
            )
            for m in range(nch_h):
                nc.tensor.matmul(
                    psh[:, m * BL : (m + 1) * BL],
                    lhsT=w1a[:, m * BL : (m + 1) * BL], rhs=xijT[:],
                    start=False, stop=False, skip_group_check=True,
                )

            # --- top-8 rounds: select nonzero-weight slots, decode (w, gid),
            # scatter w into the weighted selector Wsel[b, s] = sum_k w_k *
            # onehot(gid_k) (pair-major, per-partition-scalar DVE ops) ---
            n_rounds = max(1, -(-total_slots // TOPK))
            S = n_chunks * BL
            wsel = sb.tile([BL, S], bf16, tag="wsel")
            first = True
            tk = tkey
            for r in range(n_rounds):
                g = min(TOPK, max(1, total_slots) - r * TOPK)
                t8 = sb.tile([BL, 8], f32, tag=f"t8_{r}")
                nc.vector.max(out=t8[:], in_=tk[:])
                t8i = sb.tile([BL, 8], i32, tag=f"t8i_{r}")
                nc.vector.tensor_copy(out=t8i[:], in_=t8[:])
                gid8i = sb.tile([BL, 8], i32, tag=f"gid8i_{r}")
                nc.vector.tensor_single_scalar(
                    out=gid8i[:], in_=t8i[:], scalar=GID_BASE - 1,
                    op=mybir.AluOpType.bitwise_and,
                )
                gid8 = sb.tile([BL, 8], f32, tag=f"gid8_{r}")
                nc.vector.tensor_copy(out=gid8[:], in_=gid8i[:])
                w8i = sb.tile([BL, 8], i32, tag=f"w8i_{r}")
                nc.vector.tensor_single_scalar(
                    out=w8i[:], in_=t8i[:], scalar=9,
                    op=mybir.AluOpType.arith_shift_right,
                )
                w8 = sb.tile([BL, 8], f32, tag=f"w8_{r}")
                nc.vector.tensor_copy(out=w8[:], in_=w8i[:])

                for k in range(g):
                    if first:
                        nc.vector.tensor_scalar(
                            out=wsel[:], in0=iotaf[:, 0:S],
                            scalar1=gid8[:, k : k + 1],
                            scalar2=w8[:, k : k + 1],
                            op0=mybir.AluOpType.is_equal,
                            op1=mybir.AluOpType.mult,
                        )
                        first = False
                    else:
                        ohw = sb.tile([BL, S], bf16, tag="ohw")
                        nc.vector.tensor_scalar(
                            out=ohw[:], in0=iotaf[:, 0:S],
                            scalar1=gid8[:, k : k + 1],
                            scalar2=w8[:, k : k + 1],
                            op0=mybir.AluOpType.is_equal,
                            op1=mybir.AluOpType.mult,
                        )
                        nc.vector.tensor_add(
                            out=wsel[:], in0=wsel[:], in1=ohw[:]
                        )
                if r + 1 < n_rounds:
                    tk2 = sb.tile([BL, sj], f32, tag=f"tkey_{r + 1}")
                    nc.vector.match_replace(
                        out=tk2[:], in_to_replace=t8[:], in_values=tk[:],
                        imm_value=0.0,
                    )
                    tk = tk2

            # --- xcnT[d, b] = sum_s xsmall[s, d] * Wsel[b, s]: transpose
            # Wsel chunks on the PE, then gather-matmul against the
            # SBUF-resident compact x table ---
            psx = ps.tile([BL, BL], f32, tag="psx")
            for m in range(n_chunks):
                pswt = ps.tile([BL, BL], bf16, tag=f"pswt_{m}")
                nc.tensor.transpose(
                    out=pswt[:], in_=wsel[:, m * BL : (m + 1) * BL],
                    identity=ident[:],
                )
                wselT = sb.tile([BL, BL], bf16, tag=f"wselT_{m}")
                nc.scalar.copy(out=wselT[:], in_=pswt[:])
                nc.tensor.matmul(
                    psx[:], lhsT=xsm(m), rhs=wselT[:],
                    start=(m == 0), stop=(m == n_chunks - 1),
                    skip_group_check=True,
                )
            xcnT = sb.tile([BL, BL], bf16, tag="xcnT")
            nc.scalar.copy(out=xcnT[:], in_=psx[:])

            # --- MLP: out = relu([xij, xcn] @ W1 + b1) @ W2 + b2 ---
            for m in range(nch_h):
                nc.tensor.matmul(
                    psh[:, m * BL : (m + 1) * BL],
                    lhsT=w1b[:, m * BL : (m + 1) * BL], rhs=xcnT[:],
                    start=False, stop=True, skip_group_check=True,
                )
            hr = sb.tile([BL, DH], bf16, tag="hr")
            nc.scalar.activation(
                out=hr[:], in_=psh[:],
                func=mybir.ActivationFunctionType.Relu,
            )
            psr = ps.tile([1, BL], f32, tag="psr")
            for m in range(nch_h):
                nc.tensor.matmul(
                    psr[:], lhsT=w2c[:, m : m + 1],
                    rhs=hr[:, m * BL : (m + 1) * BL],
                    start=(m == 0), stop=(m == nch_h - 1),
                    skip_group_check=True,
                )
            res = sb.tile([1, BL], f32, tag="res")
            nc.scalar.activation(
                out=res[:], in_=psr[:],
                func=mybir.ActivationFunctionType.Identity,
                bias=b2b[0:1, :],
            )
            nc.sync.dma_start(out_d[:], res[:])

    nc.compile()
    return nc


def _prepare(x, edge_index, tar_ei, W1, b1, W2, b2):
    import ml_dtypes

    bf16 = ml_dtypes.bfloat16

    e0 = np.asarray(edge_index[0]).astype(np.int64)
    e1 = np.asarray(edge_index[1]).astype(np.int64)
    src = np.concatenate([e0, e1])
    dst = np.concatenate([e1, e0])
    tar_i = np.asarray(tar_ei[0]).astype(np.int64)
    tar_j = np.asarray(tar_ei[1]).astype(np.int64)

    ni = _padded_rows(src, dst, tar_i, sentinel=-1.0)
    nj = _padded_rows(src, dst, tar_j, sentinel=-2.0)
    si, sj = ni.shape[1], nj.shape[1]
    assert si <= 400 and sj <= 16384, (si, sj)

    x = np.ascontiguousarray(np.asarray(x, dtype=np.float32))
    w1 = np.asarray(W1, dtype=np.float32)

    # Per-core planning: intersection counts (device recomputes these), the
    # per-core compact int16 id space for the equality pass, the compact
    # gather-id space for the common-neighbor x rows, banded row placement,
    # and sizing. Rows ship sorted with pads evenly interleaved so every
    # match lies in a narrow diagonal band (host-verified exact).
    PAD_I, PAD_J = 32001, 32002
    total_slots = 1
    n_chunks = 1
    dmin, dmax = 0, 0
    cores = []
    for ci in range(N_CORES):
        sl = slice(ci * BL, (ci + 1) * BL)
        nic, njc = ni[sl], nj[sl]
        # order-preserving compact codes of the real node ids (pads < 0)
        uni = np.unique(np.concatenate([nic[nic >= 0], njc[njc >= 0]]))
        assert len(uni) < 32000, len(uni)

        def _place(rows, s, padv):
            out = np.full((BL, s), padv, np.int16)
            for b in range(BL):
                vals = np.sort(rows[b][rows[b] >= 0])
                deg = len(vals)
                if deg:
                    slots = ((np.arange(deg) + 0.5) * s / deg).astype(np.int64)
                    out[b, slots] = np.searchsorted(uni, vals)
            return out

        ni_sl = _place(nic, si, PAD_I)
        nj_sl = _place(njc, sj, PAD_J)
        eqm = nj_sl[:, :, None] == ni_sl[:, None, :]  # [BL, sj, si]
        cm = eqm.sum(-1)
        total_slots = max(total_slots, int((cm > 0).sum(-1).max()))
        bb, qq, pp = np.nonzero(eqm)
        if len(qq):
            dmin = min(dmin, int((pp - qq).min()))
            dmax = max(dmax, int((pp - qq).max()))
        rr, qq2 = np.nonzero(cm > 0)
        cn_codes = np.unique(nj_sl[rr, qq2])
        cn_nodes = uni[cn_codes.astype(np.int64)].astype(np.int64)
        gid_of = {int(c): g + 1 for g, c in enumerate(cn_codes)}
        njg = np.zeros((BL, sj), np.float32)
        njg[rr, qq2] = np.array(
            [gid_of[int(c)] for c in nj_sl[rr, qq2]], np.float32
        )
        n_rows = len(cn_nodes) + 1
        n_chunks = max(n_chunks, -(-n_rows // BL))
        cores.append((njg, cn_nodes, ni_sl, nj_sl))
    dband = dmax - dmin + 1
    assert n_chunks * BL <= GID_BASE
    # per-pair weight c ranges over counts <= si; key = c*512+gid needs to
    # stay exactly representable in f32 (c*512+511 <= 2^24)
    assert si * GID_BASE + GID_BASE - 1 < (1 << 24)

    lay, widths = _layout(dband, sj, n_chunks)

    in_maps = []
    for ci in range(N_CORES):
        sl = slice(ci * BL, (ci + 1) * BL)
        njg, cn_nodes, ni_sl, nj_sl = cores[ci]
        blocks = {
            "hoti": np.zeros((BL, widths["hoti"]), np.int16),
            "hotf": np.zeros((BL, widths["hotf"]), np.float32),
            "cbw": np.zeros((BL, widths["cbw"]), np.float32),
            "cbx": np.zeros((BL, widths["cbx"]), np.float32),
        }

        def put(name, val):
            blk, off, w = lay[name]
            blocks[blk][:, off : off + w] = val

        nis = np.full((BL, sj + dband - 1), PAD_I, np.int16)
        lo = max(0, dmin)
        hi = min(si, sj + dmax)
        nis[:, lo - dmin : hi - dmin] = ni_sl[:, lo:hi]
        put("nis", nis)
        put("nj", nj_sl)
        put("njg", njg)
        put("b2b", np.float32(np.asarray(b2).reshape(-1)[0]))
        put("pidx", np.arange(BL, dtype=np.float32)[:, None])
        _, _, iw = lay["iotaf"]
        put("iotaf", np.arange(iw, dtype=np.float32)[None, :])
        put("b1c", np.asarray(b1, np.float32).reshape(DH // BL, BL).T)
        put("w2c", np.asarray(W2, np.float32).reshape(DH // BL, BL).T)
        put("w1a", w1[0:D])
        put("w1b", w1[D : 2 * D])
        put("xiT", x[tar_i[sl]].T)
        put("xjT", x[tar_j[sl]].T)
        xsmv = np.zeros((n_chunks * BL, D), np.float32)
        xsmv[1 : 1 + len(cn_nodes)] = x[cn_nodes]
        blk, off, w = lay["xsm"]
        for m in range(n_chunks):
            blocks["cbx"][:, off + m * BL : off + (m + 1) * BL] = xsmv[
                m * BL : (m + 1) * BL
            ]
        in_maps.append({
            "hoti": blocks["hoti"],
            "hotf": blocks["hotf"],
            "cbw": blocks["cbw"].astype(bf16),
            "cbx": blocks["cbx"].astype(bf16),
        })
    return in_maps, dband, sj, total_slots, n_chunks


def kernel(x, edge_index, tar_ei, W1, b1, W2, b2):
    from concourse.bass_utils import run_bass_kernel_spmd

    in_maps, dband, sj, total_slots, n_chunks = _prepare(
        x, edge_index, tar_ei, W1, b1, W2, b2
    )

    key = (dband, sj, total_slots, n_chunks)
    if key not in _compiled_cache:
        _compiled_cache[key] = _build_bass(dband, sj, total_slots, n_chunks)
    nc = _compiled_cache[key]

    res = run_bass_kernel_spmd(nc, in_maps, list(range(N_CORES)))
    return np.concatenate(
        [res.results[ci]["out"].reshape(BL) for ci in range(N_CORES)]
    ).astype(np.float32)
